# revision 68
# baseline (speedup 1.0000x reference)
"""GAT (2-layer graph attention + pair scoring) on 8 TRN2 NeuronCores.

Sharding: destination-node rows (4096/8=512 per core). Per layer, each core
computes Wh for its rows (bf16 matmuls), AllGathers a scaled payload, then
computes masked attention for its 512 rows against all 4096 sources.

Key algebra: exp(leaky_relu(s)) with s = e_src_i + e_dst_j factors as
  p_i * q_j * max(a_i*b_j, 1),  a=exp(.8 e_src), b=exp(.8 e_dst),
  p=exp(.2 e_src), q=exp(.2 e_dst)
and p_i cancels between softmax numerator and denominator. So the N^2 stage
needs NO transcendentals: one dual-op tensor_scalar (mult+max, 2x DVE mode)
and one mask tensor_tensor per (j, head). The mask multiplies for the 8
c-blocks of one (m, head) are fused into ONE [128, 8x512] strided TT on DVE
(amortizes the per-op init), except for GP_HEADS whose mask TTs run as
small ops on the otherwise-idle GPSIMD. The softmax denominator rides as an
extra q-column in the matmul's stationary operand.

AllGather-1 is split into 4 per-m chunks pipelined behind phase-A payload
computation; phase B consumes chunks in arrival order (j = 4c + m). AG2 is
split in 2. Dummy bf16 matmuls re-warm the PE HAM clock gate (the PE drops
to 1.2 GHz after any >=3.4us idle window and only returns to 2.4 GHz after a
sustained-busy window; phase B alone never re-warms it).

Final embeddings go out as ONE fused bf16 AllGather [N, 128] = [h | h@Ws^T];
pair rows come back via HBM dma_gather in 1024-index chunks spread over 4
SWDGE queues; scoring is a TT + free-dim reduce in pair-major layout.
"""

import sys

if "/opt/trn_rl_repo" not in sys.path:
    sys.path.insert(0, "/opt/trn_rl_repo")

import numpy as np
import ml_dtypes

import concourse.bacc as bacc
import concourse.tile as tile
import concourse.mybir as mybir

BF16 = mybir.dt.bfloat16
F32 = mybir.dt.float32
I16 = mybir.dt.int16
AF = mybir.ActivationFunctionType
OP = mybir.AluOpType
AX = mybir.AxisListType

N, NFEAT, NHID, NHEADS = 4096, 512, 64, 8
P = 65536
NCORES = 8
R = N // NCORES          # rows (destination nodes) per core = 512
JT = N // 128            # source j-tiles = 32
PC = P // NCORES         # pairs per core = 8192
CH = PC // 128           # pair chunks in the score layout = 64
HB = NHID + 1            # per-head AG1 block [Wh*e1 (64) | e1] = 65
AG1C = NHEADS * HB + NHEADS   # 520 + 8 trailing rb columns = 528
AG2C = NHID + 2          # [Wh2*e1 (64) | e1 | rb] = 66

# NOTE: gpsimd is kept OUT of the attention loops: its Q7 tensor ops are
# 8-30x slower than DVE (tensor_scalar ~8us, big TT ~12us for [128,4096])
# and its in-order coupling head-of-line-blocks the tensor queue.

GCHK = 1024              # indices per dma_gather call
GNC = PC // GCHK         # gather chunks per table = 8
N_WARM = 16              # bf16 dummy matmuls to re-warm the PE clock gate


def _build_nc(stage=99, iters=1):
    nc = bacc.Bacc("TRN2", target_bir_lowering=False, debug=False,
                   num_devices=NCORES, num_swdge_queues=4)

    def inp(name, shape, dt):
        return nc.dram_tensor(name, shape, dt, kind="ExternalInput").ap()

    xT = inp("xT", [NFEAT, R], BF16)           # x[rows].T  (feature-major)
    maskT = inp("maskT", [N, R], BF16)         # adj[rows].T (0/1)
    Wcat = inp("Wcat", [NFEAT, NHEADS * NHID], BF16)
    WAsrc = inp("WAsrc", [NFEAT, NHEADS], BF16)  # Wcat @ blockdiag(a_src)
    WAdst = inp("WAdst", [NFEAT, NHEADS], BF16)  # Wcat @ blockdiag(a_dst)
    hsel = inp("hsel", [NHEADS, NHEADS], F32)    # identity (head selector)
    Wout = inp("Wout", [NHEADS * NHID, NHID], BF16)
    aout2 = inp("aout2", [NHID, 2], BF16)      # col0 = a_out[:64], col1 = a_out[64:]
    WsT = inp("WsT", [NHID, NHID], F32)        # W_score.T
    ident = inp("ident", [128, 128], F32)
    selbc = inp("selbc", [NHEADS, R], BF16)     # selbc[h, m] = (m//64 == h)
    idx1 = inp("idx1", [128, PC // 16], I16)   # dma_gather 16-partition wrap
    idx2 = inp("idx2", [128, PC // 16], I16)

    scores = nc.dram_tensor("scores", [128, CH], F32,
                            kind="ExternalOutput").ap()

    rg = [list(range(NCORES))]

    with tile.TileContext(nc) as tc:
        with tc.tile_pool(name="sb", bufs=1) as sb, \
             tc.tile_pool(name="sbw", bufs=12) as sbw, \
             tc.tile_pool(name="ps", bufs=8, space="PSUM") as ps, \
             tc.tile_pool(name="dram", bufs=1, space="DRAM") as dram:

            for _it in range(iters):
                def pst(name):
                    return ps.tile([128, R], F32, tag="ps", name=name)

                # ---------- persistent loads ----------
                Wout_sb = sb.tile([128, 4, NHID], BF16, tag="Wout")
                nc.sync.dma_start(Wout_sb[:], Wout.rearrange("(k p) c -> p k c", p=128))
                aout2_sb = sb.tile([NHID, 2], BF16, tag="aout2")
                nc.sync.dma_start(aout2_sb[:], aout2[:])
                WsT_sb = sb.tile([NHID, NHID], F32, tag="WsT")
                nc.sync.dma_start(WsT_sb[:], WsT[:])
                ident_sb = sb.tile([128, 128], F32, tag="ident")
                nc.sync.dma_start(ident_sb[:], ident[:])
                selbc_sb = sb.tile([NHEADS, R], BF16, tag="selbc")
                nc.sync.dma_start(selbc_sb[:], selbc[:])
                ones_sb = sb.tile([65, 128], F32, tag="ones")
                nc.vector.memset(ones_sb[:], 1.0)

                abc_sb = sb.tile([128, NHEADS, R], BF16, tag="abc")
                ag1m_sb = [sb.tile([128, NCORES, AG1C], BF16,
                                   tag=f"ag1m{_m}", name=f"ag1m{_m}")
                           for _m in range(4)]
                rbf_sb = sb.tile([128, 4 * NCORES, NHEADS], F32, tag="rbf")
                hcatT_sb = [sb.tile([128, R], BF16, tag=f"hcatT{_g}",
                                    name=f"hcatT{_g}") for _g in range(4)]
                num_sb = sb.tile([128, 4, R], BF16, tag="num")

                ag1_in = [dram.tile([128, AG1C], BF16, tag=f"ag1in{_m}",
                                    name=f"ag1in{_m}") for _m in range(4)]
                ag1_out = [dram.tile([128 * NCORES, AG1C], BF16,
                                     tag=f"ag1out{_m}", name=f"ag1out{_m}",
                                     addr_space="Shared") for _m in range(4)]

                # ---------- Phase A: local Wh / e / exps / chunked AG1 ----------
                with tc.tile_pool(name="sbA", bufs=1) as sbA:
                    xT_sb = sbA.tile([128, 4, R], BF16, tag="xT")
                    nc.sync.dma_start(
                        xT_sb[:, 0:2, :],
                        xT[0:NFEAT // 2].rearrange("(k p) c -> p k c", p=128))
                    nc.sync.dma_start(
                        xT_sb[:, 2:4, :],
                        xT[NFEAT // 2:NFEAT].rearrange("(k p) c -> p k c", p=128))
                    # Wcat on the Activation-engine HWDGE ring so it loads in
                    # parallel with xT on the Sync ring (both gate the first
                    # Wh matmul and with it the first AG1 chunk trigger)
                    Wcat_sb = sbA.tile([128, 4, NHEADS * NHID], BF16, tag="Wcat")
                    nc.scalar.dma_start(
                        Wcat_sb[:, 0:2, :],
                        Wcat[0:NFEAT // 2].rearrange("(k p) c -> p k c", p=128))
                    nc.scalar.dma_start(
                        Wcat_sb[:, 2:4, :],
                        Wcat[NFEAT // 2:NFEAT].rearrange("(k p) c -> p k c", p=128))
                    WAsrc_sb = sbA.tile([128, 4, NHEADS], BF16, tag="WAsrc")
                    nc.sync.dma_start(WAsrc_sb[:],
                                      WAsrc.rearrange("(k p) c -> p k c", p=128))
                    WAdst_sb = sbA.tile([128, 4, NHEADS], BF16, tag="WAdst")
                    nc.sync.dma_start(WAdst_sb[:],
                                      WAdst.rearrange("(k p) c -> p k c", p=128))
                    hsel_sb = sbA.tile([NHEADS, NHEADS], F32, tag="hsel")
                    nc.sync.dma_start(hsel_sb[:], hsel[:])

                    # bulk loads not needed until phase B, issued after
                    # phase A's inputs so they don't gate the first matmuls.
                    # mask tiles arrive m-major from the host: slot m*8+c
                    # holds j-tile 4c+m, so one (m, head)'s 8 c-blocks are
                    # CONTIGUOUS (one big TT)
                    maskT_sb = sb.tile([128, JT, R], BF16, tag="maskT")
                    nc.scalar.dma_start(maskT_sb[:],
                                        maskT.rearrange("(j p) c -> p j c", p=128))
                    idx1_sb = sb.tile([128, PC // 16], I16, tag="idx1")
                    nc.scalar.dma_start(idx1_sb[:], idx1[:])
                    idx2_sb = sb.tile([128, PC // 16], I16, tag="idx2")
                    nc.scalar.dma_start(idx2_sb[:], idx2[:])

                    # per-m: Wh rows, e_dst (x @ W@a_dst, no WhT needed),
                    # exps, payload chunk, AG chunk
                    wh_sb = sbA.tile([128, 4, NHEADS * NHID], BF16, tag="wh")
                    for m in range(4):
                        wh_ps = pst(f"whps{m}")
                        for k in range(4):
                            nc.tensor.matmul(wh_ps[:],
                                             xT_sb[:, k, m * 128:(m + 1) * 128],
                                             Wcat_sb[:, k, :],
                                             start=(k == 0), stop=(k == 3))
                        nc.scalar.copy(wh_sb[:, m, :], wh_ps[:])
                        ed_ps = pst(f"ed{m}")
                        for k in range(4):
                            nc.tensor.matmul(ed_ps[:, 0:NHEADS],
                                             xT_sb[:, k, m * 128:(m + 1) * 128],
                                             WAdst_sb[:, k, :],
                                             start=(k == 0), stop=(k == 3))
                        e1f = sbA.tile([128, NHEADS], F32, tag=f"e1f{m}",
                                       name=f"e1f{m}")
                        nc.scalar.activation(e1f[:], ed_ps[:, 0:NHEADS], AF.Exp,
                                             scale=0.2)
                        pay1m = sbA.tile([128, AG1C], BF16, tag=f"pay1{m}",
                                         name=f"pay1{m}")
                        nc.scalar.activation(pay1m[:, NHEADS * HB:AG1C],
                                             ed_ps[:, 0:NHEADS], AF.Exp,
                                             scale=0.8)
                        for h in range(NHEADS):
                            nc.vector.tensor_scalar(
                                pay1m[:, h * HB:h * HB + NHID],
                                wh_sb[:, m, h * NHID:(h + 1) * NHID],
                                e1f[:, h:h + 1], None, OP.mult)
                            nc.vector.tensor_copy(
                                pay1m[:, h * HB + NHID:h * HB + NHID + 1],
                                e1f[:, h:h + 1])
                        nc.sync.dma_start(ag1_in[m][:], pay1m[:])
                        nc.gpsimd.collective_compute(
                            "AllGather", OP.bypass, replica_groups=rg,
                            ins=[ag1_in[m].opt()], outs=[ag1_out[m].opt()])

                    # abc[h] = exp(0.8*e_src_h) broadcast to all partitions:
                    # e_srcT = (W@a_src)^T x^T on 8 psum partitions, then a
                    # stride-0 one-hot lhsT replicates row h to 128 partitions
                    es_ps = ps.tile([NHEADS, R], F32, tag="ps", name="es")
                    for k in range(4):
                        nc.tensor.matmul(es_ps[:], WAsrc_sb[:, k, :],
                                         xT_sb[:, k, :],
                                         start=(k == 0), stop=(k == 3))
                    es_sb = sbA.tile([NHEADS, R], F32, tag="es_sb")
                    nc.scalar.copy(es_sb[:], es_ps[:])
                    for h in range(NHEADS):
                        ebc_ps = pst(f"ebc{h}")
                        nc.tensor.matmul(
                            ebc_ps[:],
                            hsel_sb[:, h:h + 1].to_broadcast([NHEADS, 128]),
                            es_sb[:], start=True, stop=True)
                        nc.scalar.activation(abc_sb[:, h, :], ebc_ps[:], AF.Exp,
                                             scale=0.8)

                # load the 8 trailing rb columns first (tiny DMA) so the
                # phase-B tensor_scalars don't wait on the 1.1MB bulk load
                for m in range(4):
                    nc.sync.dma_start(
                        ag1m_sb[m][:, :, NHEADS * HB:AG1C],
                        ag1_out[m][:, NHEADS * HB:AG1C].rearrange(
                            "(c p) k -> p c k", p=128))
                    nc.scalar.copy(rbf_sb[:, m * NCORES:(m + 1) * NCORES, :],
                                   ag1m_sb[m][:, :, NHEADS * HB:AG1C])
                    nc.sync.dma_start(
                        ag1m_sb[m][:, :, 0:NHEADS * HB],
                        ag1_out[m][:, 0:NHEADS * HB].rearrange(
                            "(c p) k -> p c k", p=128))

                if stage == 1:
                    dmy = sb.tile([128, CH], F32, tag="dmy", name="dmy1")
                    nc.vector.memset(dmy[:], 0.0)
                    nc.sync.dma_start(scores[:], dmy[:])
                    return nc

                # pair-gather destinations (persistent tiles)
                ag3_out = dram.tile([N, 2 * NHID], BF16, tag="ag3out",
                                    addr_space="Shared")
                g1_sb = [sb.tile([128, CH // 4, 2 * NHID], BF16,
                                 tag=f"g1_{_q}", name=f"g1_{_q}")
                         for _q in range(4)]
                g2_sb = [sb.tile([128, CH // 4, 2 * NHID], BF16,
                                 tag=f"g2_{_q}", name=f"g2_{_q}")
                         for _q in range(4)]

                # ---------- Phase B: layer-1 attention ----------
                # dummy matmuls: one continuous >=3.4us burst re-warms the
                # HAM clock gate before the phase-B matmul train. They read
                # the ag1m chunk-0 bulk load so they fire only once that DMA
                # lands — i.e. RIGHT before phase B's matmuls need the PE
                # (an early burst would decay again during the AG wait)
                if N_WARM:
                    wrm_ps = pst("warm")
                    for w in range(N_WARM):
                        nc.tensor.matmul(wrm_ps[0:65, :],
                                         ag1m_sb[0][:, 0, 0:65],
                                         ag1m_sb[0][:, 0, 0:512],
                                         start=(w == 0), stop=(w == N_WARM - 1))
                # lhsT = [Whq | q]: numerators on psum p0..63, denom on p64.
                # Per (m, head): 8 dual-op tensor_scalars build the unmasked
                # weights for all 8 c-blocks into one [128, 8, 512] tile,
                # then ONE big contiguous TT applies the mask (DVE heads) or
                # 8 small gpsimd TTs do (GP_HEADS).
                hp_ps = [ps.tile([65, R], F32, tag="ps", name=f"hp{h}")
                         for h in range(NHEADS)]
                den_st = sb.tile([65, NHEADS, R], F32, tag="den_st")
                with tc.tile_pool(name="sbB", bufs=4) as sbB:
                    def b_group(m, h, is_start, is_stop):
                        u8 = sbB.tile([128, NCORES, R], BF16, tag="u8",
                                      bufs=2, name=f"u8_{m}_{h}")
                        for c in range(NCORES):
                            nc.vector.tensor_scalar(
                                u8[:, c, :], abc_sb[:, h, :],
                                rbf_sb[:, m * NCORES + c, h:h + 1],
                                1.0, OP.mult, OP.max)
                        t8 = sbB.tile([128, NCORES, R], BF16, tag="t8",
                                      bufs=6, name=f"t8_{m}_{h}")
                        for half in range(2):
                            hs = slice(half * (NCORES // 2),
                                       (half + 1) * (NCORES // 2))
                            nc.vector.tensor_tensor(
                                t8[:, hs, :], u8[:, hs, :],
                                maskT_sb[:, m * NCORES + half * (NCORES // 2):
                                         m * NCORES + (half + 1) * (NCORES // 2),
                                         :],
                                OP.mult)
                            for c in range(half * (NCORES // 2),
                                           (half + 1) * (NCORES // 2)):
                                nc.tensor.matmul(
                                    hp_ps[h][:],
                                    ag1m_sb[m][:, c, h * HB:h * HB + HB],
                                    t8[:, c, :], start=(is_start and c == 0),
                                    stop=(is_stop and c == NCORES - 1))

                    for m in range(3):
                        for h in range(NHEADS):
                            b_group(m, h, m == 0, False)
                    # last m-block per head, with that head's PSUM
                    # evacuation streamed while later heads still compute
                    for h in range(NHEADS):
                        b_group(3, h, False, True)
                        nc.scalar.copy(den_st[64:65, h, :],
                                       hp_ps[h][64:65, :])
                        if h % 2 == 0:
                            nc.scalar.copy(num_sb[0:64, h // 2, :],
                                           hp_ps[h][0:64, :])
                        else:
                            nc.vector.tensor_copy(num_sb[64:128, h // 2, :],
                                                  hp_ps[h][0:64, :])

                if stage == 2:
                    dmy = sb.tile([128, CH], F32, tag="dmy", name="dmy2")
                    nc.vector.memset(dmy[:], 0.0)
                    nc.sync.dma_start(scores[:], dmy[:])
                    return nc
                # normalize + elu -> hcatT [512_hd, 512_i] bf16
                with tc.tile_pool(name="sbE", bufs=1) as sbE:
                    rin_sb = sbE.tile([NHEADS, R], F32, tag="rin")
                    nc.sync.dma_start(rin_sb[:], den_st[64:65, :, :])
                    rcp_sb = sbE.tile([NHEADS, R], F32, tag="rcp")
                    nc.vector.reciprocal(rcp_sb[:], rin_sb[:])
                    rcpb_sb = sbE.tile([NHEADS, R], BF16, tag="rcpb")
                    nc.vector.tensor_copy(rcpb_sb[:], rcp_sb[:])

                    # per-chunk tiles: norm+elu stream into phase C's
                    # accumulating matmuls chunk-by-chunk
                    with tc.tile_pool(name="sbG", bufs=2) as sbG:
                        for g in range(4):
                            rbc_ps = pst(f"rbc{g}")
                            nc.tensor.matmul(rbc_ps[:],
                                             selbc_sb[:, g * 128:(g + 1) * 128],
                                             rcpb_sb[:], start=True, stop=True)
                            xbg = sbG.tile([128, R], BF16, tag="xbg",
                                           name=f"xbg{g}")
                            nc.vector.tensor_tensor(xbg[:], num_sb[:, g, :],
                                                    rbc_ps[:], OP.mult)
                            # elu(x) = relu(x) + exp(min(x,0)) - 1
                            tmin = sbG.tile([128, R], BF16, tag="tming",
                                            name=f"tmin{g}")
                            nc.vector.tensor_scalar(tmin[:], xbg[:], 0.0, None,
                                                    OP.min)
                            texp = sbG.tile([128, R], BF16, tag="texpg",
                                            name=f"texp{g}")
                            nc.scalar.activation(texp[:], tmin[:], AF.Exp)
                            trel = sbG.tile([128, R], BF16, tag="trelg",
                                            name=f"trel{g}")
                            nc.vector.tensor_scalar(trel[:], xbg[:], 0.0, 1.0,
                                                    OP.max, OP.subtract)
                            nc.vector.tensor_tensor(hcatT_sb[g][:], texp[:],
                                                    trel[:], OP.add)

                # ---------- Phase C: layer-2 ingredients + chunked AG2 ----------
                ag2_in = [dram.tile([256, AG2C], BF16, tag=f"ag2in{_q}",
                                    name=f"ag2in{_q}") for _q in range(2)]
                ag2_out = [dram.tile([256 * NCORES, AG2C], BF16,
                                     tag=f"ag2out{_q}", name=f"ag2out{_q}",
                                     addr_space="Shared") for _q in range(2)]
                wh2T_sb = sb.tile([NHID, R], BF16, tag="wh2Tsb")
                a2bc_sb = sb.tile([128, R], BF16, tag="a2bcsb")
                with tc.tile_pool(name="sbC", bufs=1) as sbC:
                    wh2T_ps = ps.tile([NHID, R], F32, tag="ps", name="wh2T")
                    for k in range(4):
                        nc.tensor.matmul(wh2T_ps[:], Wout_sb[:, k, :],
                                         hcatT_sb[k][:],
                                         start=(k == 0), stop=(k == 3))
                    nc.scalar.copy(wh2T_sb[:], wh2T_ps[:])

                    a2e_ps = pst("a2e")
                    nc.tensor.matmul(a2e_ps[:],
                                     aout2_sb[:, 0:1].to_broadcast([NHID, 128]),
                                     wh2T_sb[:], start=True, stop=True)
                    nc.scalar.activation(a2bc_sb[:], a2e_ps[:], AF.Exp, scale=0.8)

                    # AG2 payload block: [Wh2*e1 (64) | e1 | rb]
                    pay2_sb = sbC.tile([128, 4, AG2C], BF16, tag="pay2")
                    for m in range(4):
                        wh2_ps = ps.tile([128, NHID], F32, tag="ps",
                                         name=f"wh2_{m}")
                        for k in range(4):
                            nc.tensor.matmul(wh2_ps[:],
                                             hcatT_sb[k][:, m * 128:(m + 1) * 128],
                                             Wout_sb[:, k, :],
                                             start=(k == 0), stop=(k == 3))
                        ed2_ps = ps.tile([128, 1], F32, tag="ps", name=f"ed2_{m}")
                        nc.tensor.matmul(ed2_ps[:],
                                         wh2T_sb[:, m * 128:(m + 1) * 128],
                                         aout2_sb[:, 1:2], start=True, stop=True)
                        e2f = sbC.tile([128, 4], F32, tag="e2f", bufs=4,
                                       name=f"e2f_{m}")
                        nc.scalar.activation(e2f[:, 0:1], ed2_ps[:], AF.Exp,
                                             scale=0.2)
                        nc.vector.tensor_copy(pay2_sb[:, m, NHID:NHID + 1],
                                              e2f[:, 0:1])
                        nc.scalar.activation(pay2_sb[:, m, NHID + 1:NHID + 2],
                                             ed2_ps[:], AF.Exp, scale=0.8)
                        nc.vector.tensor_scalar(pay2_sb[:, m, 0:NHID],
                                                wh2_ps[:], e2f[:, 0:1],
                                                None, OP.mult)
                        if m % 2 == 1:
                            q = m // 2
                            nc.sync.dma_start(
                                ag2_in[q][:].rearrange("(m p) c -> p m c", p=128),
                                pay2_sb[:, 2 * q:2 * q + 2, :])
                            nc.gpsimd.collective_compute(
                                "AllGather", OP.bypass, replica_groups=rg,
                                ins=[ag2_in[q].opt()], outs=[ag2_out[q].opt()])

                ag2_sb = [sb.tile([128, 2 * NCORES, AG2C], BF16,
                                  tag=f"ag2sb{_q}", name=f"ag2sb{_q}")
                          for _q in range(2)]
                rb2f_sb = sb.tile([128, 2 * 2 * NCORES, 1], F32, tag="rb2f")
                # rb column first (tiny DMA) so phase-D tensor_scalars don't
                # wait on the bulk chunk load
                for q in range(2):
                    nc.sync.dma_start(
                        ag2_sb[q][:, :, NHID + 1:NHID + 2],
                        ag2_out[q][:, NHID + 1:NHID + 2].rearrange(
                            "(s p) k -> p s k", p=128))
                    nc.scalar.copy(
                        rb2f_sb[:, q * 2 * NCORES:(q + 1) * 2 * NCORES, :],
                        ag2_sb[q][:, :, NHID + 1:NHID + 2])
                    nc.sync.dma_start(
                        ag2_sb[q][:, :, 0:NHID + 1],
                        ag2_out[q][:, 0:NHID + 1].rearrange(
                            "(s p) k -> p s k", p=128))

                if stage == 3:
                    dmy = sb.tile([128, CH], F32, tag="dmy", name="dmy3")
                    nc.vector.memset(dmy[:], 0.0)
                    nc.sync.dma_start(scores[:], dmy[:])
                    return nc

                # ---------- Phase D: layer-2 attention ----------
                # grouped like phase B: per (q, hm) one [128, 8, 512] u-tile,
                # one big mask TT (the last group's TTs go to gpsimd)
                hp2_ps = ps.tile([65, R], F32, tag="ps", name="hp2")
                with tc.tile_pool(name="sbB2", bufs=2) as sbB2:
                    nit = 0
                    for q in range(2):
                        for hm in range(2):
                            g = 2 * q + hm
                            u8 = sbB2.tile([128, NCORES, R], BF16, tag="u8")
                            for c in range(NCORES):
                                s = 2 * c + hm
                                nc.vector.tensor_scalar(
                                    u8[:, c, :], a2bc_sb[:],
                                    rb2f_sb[:, q * 2 * NCORES + s, :],
                                    1.0, OP.mult, OP.max)
                            t8 = sbB2.tile([128, NCORES, R], BF16, tag="t8")
                            nc.vector.tensor_tensor(
                                t8[:], u8[:],
                                maskT_sb[:, g * NCORES:(g + 1) * NCORES, :],
                                OP.mult)
                            for c in range(NCORES):
                                s = 2 * c + hm
                                nc.tensor.matmul(
                                    hp2_ps[:], ag2_sb[q][:, s, 0:NHID + 1],
                                    t8[:, c, :], start=(nit == 0),
                                    stop=(nit == JT - 1))
                                nit += 1

                if stage == 4:
                    dmy = sb.tile([128, CH], F32, tag="dmy", name="dmy4")
                    nc.vector.memset(dmy[:], 0.0)
                    nc.sync.dma_start(scores[:], dmy[:])
                    return nc
                hfT_sb = sb.tile([NHID, R], F32, tag="hfT")
                with tc.tile_pool(name="sbD", bufs=1) as sbD:
                    rcp2_sb = sbD.tile([65, R], F32, tag="rcp2")
                    nc.vector.reciprocal(rcp2_sb[64:65, :], hp2_ps[64:65, :])
                    rbc2_ps = ps.tile([NHID, R], F32, tag="ps", name="rbc2")
                    nc.tensor.matmul(rbc2_ps[:], ones_sb[64:65, 0:NHID],
                                     rcp2_sb[64:65, :], start=True, stop=True)
                    num2_sb = sbD.tile([NHID, R], F32, tag="num2")
                    nc.scalar.copy(num2_sb[:], hp2_ps[0:NHID, :])
                    xn2_sb = sbD.tile([NHID, R], F32, tag="xn2")
                    nc.vector.tensor_tensor(xn2_sb[:], num2_sb[:],
                                            rbc2_ps[:], OP.mult)
                    # elu in f32
                    tmin2 = sbD.tile([NHID, R], F32, tag="tmin2")
                    nc.vector.tensor_scalar(tmin2[:], xn2_sb[:], 0.0, None, OP.min)
                    texp2 = sbD.tile([NHID, R], F32, tag="texp2")
                    nc.scalar.activation(texp2[:], tmin2[:], AF.Exp)
                    trel2 = sbD.tile([NHID, R], F32, tag="trel2")
                    nc.vector.tensor_scalar(trel2[:], xn2_sb[:], 0.0, 1.0, OP.max,
                                            OP.subtract)
                    nc.vector.tensor_tensor(hfT_sb[:], texp2[:], trel2[:], OP.add)

                # ---------- Phase E: H2 = h @ Ws^T, transpose h, AG3 ----------
                # single fused collective: cols [0:64] = h rows, [64:128] = h@Ws^T
                ag3_in = dram.tile([R, 2 * NHID], BF16, tag="ag3in")
                with tc.tile_pool(name="sbF", bufs=1) as sbF:
                    ag3_sb = sbF.tile([128, 4, 2 * NHID], BF16, tag="ag3")
                    for m in range(4):
                        h2_ps = ps.tile([128, NHID], F32, tag="ps", name=f"h2_{m}")
                        nc.tensor.matmul(h2_ps[:], hfT_sb[:, m * 128:(m + 1) * 128],
                                         WsT_sb[:], start=True, stop=True)
                        nc.scalar.copy(ag3_sb[:, m, NHID:2 * NHID], h2_ps[:])
                        hf_ps = ps.tile([128, NHID], F32, tag="ps", name=f"hf_{m}")
                        nc.tensor.transpose(hf_ps[:],
                                            hfT_sb[:, m * 128:(m + 1) * 128],
                                            ident_sb[0:NHID, 0:NHID])
                        nc.scalar.copy(ag3_sb[:, m, 0:NHID], hf_ps[:])

                    nc.sync.dma_start(
                        ag3_in[:].rearrange("(m p) c -> p m c", p=128), ag3_sb[:])
                    nc.gpsimd.collective_compute(
                        "AllGather", OP.bypass, replica_groups=rg,
                        ins=[ag3_in.opt()], outs=[ag3_out.opt()])

                    if stage == 5:
                        dmy = sbF.tile([128, CH], F32, tag="dmy", name="dmy5")
                        nc.vector.memset(dmy[:], 0.0)
                        nc.sync.dma_start(scores[:], dmy[:])
                        return nc

                    # ---------- Phase F: pair gather + bilinear score ----------
                    for c in range(GNC):
                        q, cq = c // (GNC // 4), c % (GNC // 4)
                        cc = slice(cq * (GCHK // 128), (cq + 1) * (GCHK // 128))
                        ci = slice(c * (GCHK // 16), (c + 1) * (GCHK // 16))
                        nc.gpsimd.dma_gather(
                            g1_sb[q][:, cc, :], ag3_out[:], idx1_sb[:, ci],
                            GCHK, GCHK, 2 * NHID, queue_num=(2 * c) % 4)
                        nc.gpsimd.dma_gather(
                            g2_sb[q][:, cc, :], ag3_out[:], idx2_sb[:, ci],
                            GCHK, GCHK, 2 * NHID, queue_num=(2 * c + 1) % 4)

                    if stage == 6:
                        dmy = sbF.tile([128, CH], F32, tag="dmy", name="dmy6")
                        nc.vector.memset(dmy[:], 0.0)
                        nc.sync.dma_start(scores[:], dmy[:])
                        return nc
                    # quarters 3,2 first (wave B, Tile-tracked waits); their
                    # completion implies wave A drained (per-queue FIFO), so
                    # quarters 1,0 are safe afterwards on the in-order DVE
                    sc_sb = sbF.tile([128, CH], F32, tag="sc")
                    for q in (3, 2, 1, 0):
                        prod_sb = sbF.tile([128, CH // 4, NHID], BF16,
                                           tag=f"prod{q}", name=f"prod{q}")
                        nc.vector.tensor_tensor(prod_sb[:],
                                                g1_sb[q][:, :, 0:NHID],
                                                g2_sb[q][:, :, NHID:2 * NHID],
                                                OP.mult)
                        nc.vector.tensor_reduce(
                            sc_sb[:, q * (CH // 4):(q + 1) * (CH // 4)],
                            prod_sb[:], AX.X, OP.add)
                    nc.sync.dma_start(scores[:], sc_sb[:])

    return nc


def _make_in_maps(x, adj, W_heads, a_heads, W_out, a_out, W_score,
                  pair1_idx, pair2_idx):
    bf = ml_dtypes.bfloat16
    x = np.asarray(x, dtype=np.float32)
    adj = np.asarray(adj, dtype=np.float32)
    W_heads = np.asarray(W_heads, dtype=np.float32)
    a_heads = np.asarray(a_heads, dtype=np.float32)
    W_out = np.asarray(W_out, dtype=np.float32)
    a_out = np.asarray(a_out, dtype=np.float32)
    W_score = np.asarray(W_score, dtype=np.float32)
    pair1_idx = np.asarray(pair1_idx, dtype=np.int32)
    pair2_idx = np.asarray(pair2_idx, dtype=np.int32)

    Wcat = np.concatenate([W_heads[h] for h in range(NHEADS)], axis=1)
    Wcat = np.ascontiguousarray(Wcat, dtype=np.float32)
    Asrc = np.zeros((NHEADS * NHID, NHEADS), dtype=np.float32)
    Adst = np.zeros((NHEADS * NHID, NHEADS), dtype=np.float32)
    for h in range(NHEADS):
        Asrc[h * NHID:(h + 1) * NHID, h] = a_heads[h, :NHID]
        Adst[h * NHID:(h + 1) * NHID, h] = a_heads[h, NHID:]
    WAsrc = Wcat @ Asrc
    WAdst = Wcat @ Adst
    hsel = np.eye(NHEADS, dtype=np.float32)
    Wout_bf = W_out.astype(bf)
    aout2 = np.stack([a_out[:NHID], a_out[NHID:]], axis=1).astype(bf)
    WsT = np.ascontiguousarray(W_score.T, dtype=np.float32)
    ident = np.eye(128, dtype=np.float32)
    selbc = np.zeros((NHEADS, R), dtype=np.float32)
    for h in range(NHEADS):
        selbc[h, h * NHID:(h + 1) * NHID] = 1.0

    # dma_gather slot i lands at dst[i % 128, (i // GCHK) * 8 + (i % GCHK) // 128]
    # permute so dst[p, ch] = pair p * CH + ch (the layout the unpack expects),
    # then wrap in the 16-partition index layout replicated over 8 groups
    i_arr = np.arange(PC)
    gperm = (i_arr % 128) * CH + (i_arr // GCHK) * (GCHK // 128) \
        + (i_arr % GCHK) // 128

    def gidx(ids):
        g = ids[gperm].astype(np.int16)
        return np.ascontiguousarray(
            np.tile(g.reshape(PC // 16, 16).T, (8, 1)))

    # mask tile permutation: device slot m*8+c holds j-tile 4c+m
    mperm = np.array([4 * (s % 8) + s // 8 for s in range(JT)])

    in_maps = []
    for c in range(NCORES):
        rows = slice(c * R, (c + 1) * R)
        mT = np.ascontiguousarray(adj[rows].T)
        mT = np.ascontiguousarray(
            mT.reshape(JT, 128, R)[mperm].reshape(N, R))
        in_maps.append(dict(
            xT=np.ascontiguousarray(x[rows].T).astype(bf),
            maskT=mT.astype(bf),
            Wcat=Wcat.astype(bf), WAsrc=WAsrc.astype(bf),
            WAdst=WAdst.astype(bf), hsel=hsel,
            Wout=Wout_bf, aout2=aout2,
            WsT=WsT, ident=ident, selbc=selbc.astype(bf),
            idx1=gidx(pair1_idx[c * PC:(c + 1) * PC]),
            idx2=gidx(pair2_idx[c * PC:(c + 1) * PC]),
        ))
    return in_maps


_CACHE = {}


def _get_compiled(stage=99, iters=1):
    key = f"nc{stage}_{iters}"
    if key not in _CACHE:
        nc = _build_nc(stage, iters)
        nc.compile()
        _CACHE[key] = nc
    return _CACHE[key]


def kernel(**inputs):
    from concourse.bass_utils import run_bass_kernel_spmd

    nc = _get_compiled()
    in_maps = _make_in_maps(**inputs)
    res = run_bass_kernel_spmd(nc, in_maps, core_ids=list(range(NCORES)))
    out = np.concatenate(
        [np.asarray(res.results[c]["scores"], dtype=np.float32).reshape(PC)
         for c in range(NCORES)])
    return out
